# revision 24
# baseline (speedup 1.0000x reference)
"""Trainium2 Bass kernel for nn_DictNet loss (8-core SPMD), v11.

v11 = v10 + coalesced collective staging + fp8 x
------------------------------------------------
* The AllGather staging write is ONE DMA (single [128, NFC, R] y^T
  tile) and each rotated remote block loads with ONE DMA instead of
  one per feature chunk: ~15 fixed DMA costs come off the serialized
  post-collective tail.
* x is shipped fp8 for the (Lx)^T matmul lhsT (the direct x^T
  subtraction stays bf16), halving the resident x load.

v10 = v9b + fp8-resident D stream
---------------------------------
* D tiles stay fp8 in SBUF (no DMA cast): SBUF ingress for the D
  stream halves again and the prefetch ring deepens (12 x 720 KB). The
  AXPY reads the fp8 tiles directly (DVE converts in-pipe); its bf16
  accumulator is unchanged. D DMAs stay on the GpSimd (SWDGE) queue so
  x/weight loads on the HWDGE queue aren't serialized behind them.

v9b = v8 + fp8 AllGather payload
--------------------------------
* y_hat^T is written in fp8e4m3: the AllGather payload, the rotated
  y^T tiles and both gram operands halve. sn rides the collective as a
  /32-scaled fp8 row, un-scaled by a 32-valued broadcast lhsT; the own
  (row-side) sn stays fp32-exact.

v8 = v7 + host-transposed D (direct L^T production)
---------------------------------------------------
* D is shipped pre-transposed per core as D^T[m, k, r] so the AXPY
  produces L^T chunks [128 m, R] directly: the whole identity-matmul
  L -> L^T fold, its PSUM traffic and the ACT copy back to SBUF are
  deleted. The y^T matmuls consume the AXPY output tile as rhs as-is.
* k is outermost in the tile free dims, so every AXPY term reads a
  contiguous 512-wide bf16 row (16-bit 2x DVE mode eligible), instead
  of the stride-11 reads of v6/v7.

v7 = v6 + fp8 D in HBM
----------------------
* D is pre-scaled by a power of two (folded into cn, so the math is
  unchanged) and shipped to the device as fp8e4m3: HBM reads of the D
  stream halve vs bf16 (quarter vs the original fp32). The SWDGE DMA
  casts fp8 -> bf16 on the way into SBUF, so everything downstream is
  identical to v6.


Math restructuring (same as v1)
-------------------------------
  Cn    = C / ||C||                      (tiny, host)
  L     = einsum('nmk,k->nm', D, Cn)     (memory-bound: 738 MB of D)
  y_hat = x - L @ x
  d     = pairwise distance matrix of y_hat rows   [N, N]
  loss  = sparsity(Cn) + sum_c u_c d u_c^T - (1/(S^2*beta)) * sum_g h_g d h_g^T

v6 = v2 + bf16-cast D stream
----------------------------
* D tiles are cast fp32 -> bf16 during the DMA (SWDGE cast path): HBM reads
  are unchanged but the DVE AXPY (the hardware phase-A bottleneck) runs at
  the 2x 16-bit rate; the L^T fold is a regular matmul against a bf16
  identity so the PSUM stays fp32 (plain TRN2 ISA).

v2 performance restructure
--------------------------
* y_hat is accumulated TRANSPOSED (y^T[f, own-rows]) directly in PSUM by
  swapping the matmul operands (lhsT = x block, rhs = L^T block), which
  deletes the whole post-phase-A transpose stage.
* x is loaded once (bf16, SBUF-resident) instead of re-streamed per m-group.
* Everything downstream of y_hat is bf16: the AllGather payload, the y^T
  tiles, the gram/vu/vh matmul operands and the distance tiles. PSUM math
  stays fp32.
* All small phase-D weights are DMA'd during the D stream (front-loaded).
* The last m-group is split into two 256-wide groups so the post-DMA AXPY
  drain is short.

Sharding: D rows (node axis) split across 8 cores; y_hat^T AllGathered so
every core forms distance tiles for its own rows. Symmetry: each core only
processes JBLK = CORES/2 + 1 rotated column blocks; off-diagonal blocks are
double-counted via host-scaled weights; the j = CORES/2 block is
zero-weighted on the upper half of the cores.
"""

import math

import numpy as np

import concourse.bass as bass
import concourse.mybir as mybir
import concourse.tile as tile
from concourse import bacc
from concourse.bass_utils import run_bass_kernel_spmd

FP32 = mybir.dt.float32
BF16 = mybir.dt.bfloat16
FP8 = mybir.dt.float8e4
AF = mybir.ActivationFunctionType
OP = mybir.AluOpType

FULL_CFG = dict(N=4096, F=512, K=11, G=128, NCLS=7, CORES=8)


def _derived(cfg):
    N, F, K, G, NCLS, CORES = (
        cfg["N"], cfg["F"], cfg["K"], cfg["G"], cfg["NCLS"], cfg["CORES"])
    R = N // CORES              # rows per core
    assert R % 128 == 0 and N % 512 == 0 and F % 128 == 0
    NRC = R // 128              # 128-row chunks per core
    NMC = N // 128              # 128-col chunks (m axis)
    NFC = F // 128              # feature chunks
    XSUB = N // 128             # m sub-blocks in resident x
    JBLK = CORES // 2 + 1       # rotated col blocks each core processes
    return dict(N=N, F=F, K=K, G=G, NCLS=NCLS, CORES=CORES, R=R, NRC=NRC,
                NMC=NMC, NFC=NFC, XSUB=XSUB, JBLK=JBLK)


def build(cfg, reps=1, stage="full", chained=False):
    """Build the SPMD kernel (one NEFF, runs on all cores).

    reps > 1 repeats the whole computation serially (timing probe).
    stage: "dma" = D loads only, "axpy" = + AXPY, "A" = phases A+B,
    "AG"/"simAG" = + collective (simAG fakes it), "sim" = full with faked
    collective (for TimelineSim), "full" = everything.
    """
    c = _derived(cfg)
    N, F, K, G, NCLS = c["N"], c["F"], c["K"], c["G"], c["NCLS"]
    CORES, R, NRC, NMC = c["CORES"], c["R"], c["NRC"], c["NMC"]
    NFC, XSUB, JBLK = c["NFC"], c["XSUB"], c["JBLK"]

    nc = bacc.Bacc("TRN2", target_bir_lowering=False, debug=False,
                   num_devices=CORES)

    # ---- I/O ----
    # D^T per core: Dsh[m, k, r] = D[r, m, k] (host pre-transposed)
    Dsh = nc.dram_tensor("Dsh", [N, K, R], FP8, kind="ExternalInput")
    x_in = nc.dram_tensor("x_in", [N, F], FP8, kind="ExternalInput")
    xT_in = nc.dram_tensor("xT_own", [F, R], BF16, kind="ExternalInput")
    cnb_in = nc.dram_tensor("cnb", [128, K], FP32, kind="ExternalInput")
    uT_in = nc.dram_tensor("uT_sh", [R, NCLS], BF16, kind="ExternalInput")
    hT_in = nc.dram_tensor("hT_sh", [R, G], BF16, kind="ExternalInput")
    u_in = nc.dram_tensor("u_rot", [NCLS, JBLK, R], FP32, kind="ExternalInput")
    h_in = nc.dram_tensor("h_rot", [G, JBLK, R], FP32, kind="ExternalInput")
    dmask_in = nc.dram_tensor("dmask", [128, NRC, R], BF16, kind="ExternalInput")
    out_u = nc.dram_tensor("out_u", [NCLS, JBLK], FP32, kind="ExternalOutput")
    out_h = nc.dram_tensor("out_h", [G, JBLK], FP32, kind="ExternalOutput")

    # cross-rep serialization bounce for single-shot timing (chained=True)
    chain = nc.dram_tensor("chain", [1, 1], FP32)
    # collective bounce buffers: rows 0..F-1 = y_hat^T (own cols, fp8),
    # row F = sn/32 (fp8)
    agin = nc.dram_tensor("agin", [F + 1, R], FP8)
    agout = nc.dram_tensor("agout", [CORES, F + 1, R], FP8,
                           addr_space="Shared")

    with tile.TileContext(nc) as tc:
      for rep in range(reps):
          with tc.tile_pool(name=f"persist{rep}", bufs=1) as pp:
              cnb = pp.tile([128, K], FP32)
              nc.sync.dma_start(cnb[:], cnb_in[:])
              if chained and rep > 0:
                  # rep k's first consumer waits on rep k-1's last result:
                  # cnb[0,0] = 0*chain + cnb[0,0] forces the dependency
                  # through real dataflow without changing the value
                  cht = pp.tile([1, 1], FP32, name=f"cht{rep}")
                  nc.sync.dma_start(cht[:], chain[:])
                  nc.vector.scalar_tensor_tensor(
                      cnb[0:1, 0:1], cht[:], 0.0, cnb[0:1, 0:1],
                      OP.mult, OP.add)

              # constants: ones in bf16 (memset fp32 then cast-copy)
              ones_f = pp.tile([1, 128], FP32)
              nc.vector.memset(ones_f[:], 1.0)
              ones_row = pp.tile([1, 128], BF16)   # [1,128] lhsT broadcaster
              nc.vector.tensor_copy(ones_row[:], ones_f[:])
              t32_f = pp.tile([1, 128], FP32)
              nc.vector.memset(t32_f[:], 32.0)
              t32_row = pp.tile([1, 128], BF16)    # un-scales the fp8 sn row
              nc.vector.tensor_copy(t32_row[:], t32_f[:])
              onesc_f = pp.tile([128, 1], FP32)
              nc.vector.memset(onesc_f[:], 1.0)
              ones_col = pp.tile([128, 1], BF16)   # [128,1] column reducer
              nc.vector.tensor_copy(ones_col[:], onesc_f[:])

              # single tile holding all fc chunks: the AllGather staging
              # write is ONE DMA instead of NFC
              yT_all = pp.tile([128, NFC, R], FP8, name=f"yT_all{rep}")
              yT_own = [yT_all[:, fc, :] for fc in range(NFC)]
              sn_own = [pp.tile([128, 1], FP32, tag=f"sn{rc}",
                                name=f"sn_own{rep}_{rc}")
                        for rc in range(NRC)]
              sn_sb = pp.tile([1, R], BF16, name=f"sn_sb{rep}")
              acc_u = pp.tile([NCLS, JBLK], FP32)
              acc_h = pp.tile([G, JBLK], FP32)
              if stage not in ("full", "sim"):
                  nc.vector.memset(acc_u[:], 0.0)
                  nc.vector.memset(acc_h[:], 0.0)

              # ------------- Phase A: L^T = sum_k cn_k * D^T_k; yT -= (Lx)^T
              with (
                  tc.tile_pool(name=f"psYT{rep}", bufs=1, space="PSUM") as psYT,
              ):
                  ytpsum = [psYT.tile([128, R], FP32, tag=f"ytp{fc}",
                                      name=f"ytpsum{rep}_{fc}")
                            for fc in range(NFC)]
                  with (
                      tc.tile_pool(name=f"dA{rep}", bufs=12) as dpool,
                      tc.tile_pool(name=f"lA{rep}", bufs=3) as lpool,
                  ):
                      # software-pipelined D-tile DMA issue: the queue is
                      # FIFO, so the big x load and the small phase-B/D
                      # weights slot in behind the first D chunks instead of
                      # delaying them
                      dtile = {}
                      issued = [0]

                      def issue_d(n):
                          for _ in range(n):
                              if issued[0] >= NMC:
                                  return
                              mc = issued[0]
                              t = dpool.tile([128, K, R], FP8, tag="D")
                              nc.gpsimd.dma_start(
                                  t[:], Dsh[mc * 128:(mc + 1) * 128, :, :])
                              dtile[mc] = t
                              issued[0] += 1

                      issue_d(4)  # first chunks ahead of everything else
                      # resident x (bf16): [p, m-sub, f]
                      x_sb = pp.tile([128, XSUB, F], FP8, name=f"x_sb{rep}")
                      nc.sync.dma_start(
                          x_sb[:], x_in[:].rearrange("(s p) f -> p s f", p=128))
                      issue_d(2)
                      # small phase-B/D operands, loaded under the D stream
                      xT_sb = pp.tile([128, NFC, R], BF16, name=f"xT_sb{rep}")
                      nc.sync.dma_start(
                          xT_sb[:],
                          xT_in[:].rearrange("(fc p) n -> p fc n", p=128))
                      uT_sb = pp.tile([128, NRC, NCLS], BF16,
                                      name=f"uT_sb{rep}")
                      nc.sync.dma_start(
                          uT_sb[:],
                          uT_in[:].rearrange("(rc p) c -> p rc c", p=128))
                      hT_sb = pp.tile([128, NRC, G], BF16, name=f"hT_sb{rep}")
                      nc.sync.dma_start(
                          hT_sb[:],
                          hT_in[:].rearrange("(rc p) g -> p rc g", p=128))
                      u_sb = pp.tile([NCLS, JBLK, R], FP32, name=f"u_sb{rep}")
                      nc.sync.dma_start(u_sb[:], u_in[:])
                      h_sb = pp.tile([G, JBLK, R], FP32, name=f"h_sb{rep}")
                      nc.sync.dma_start(h_sb[:], h_in[:])
                      dmask = pp.tile([128, NRC, R], BF16, name=f"dmask{rep}")
                      nc.sync.dma_start(dmask[:], dmask_in[:])

                      junk = pp.tile([128, 1], BF16, name=f"junk{rep}")
                      for mc in range(NMC):
                          issue_d(1)
                          if stage == "dma":
                              # tiny consumer so the DMA can't be elided
                              nc.vector.tensor_copy(
                                  junk[:], dtile.pop(mc)[:, 0, 0:1])
                              continue
                          dt = dtile.pop(mc)
                          lgT = lpool.tile([128, R], BF16, tag="L",
                                           name=f"lgT{rep}_{mc}")
                          nc.vector.tensor_scalar_mul(
                              lgT[:], dt[:, 0, :], cnb[:, 0:1])
                          for k in range(1, K):
                              nc.vector.scalar_tensor_tensor(
                                  lgT[:], dt[:, k, :],
                                  cnb[:, k:k + 1], lgT[:],
                                  OP.mult, OP.add)
                          if stage == "axpy":
                              nc.vector.tensor_copy(junk[:], lgT[:, 0:1])
                              continue
                          for fc in range(NFC):
                              nc.tensor.matmul(
                                  ytpsum[fc][:],
                                  lhsT=x_sb[:, mc, fc * 128:(fc + 1) * 128],
                                  rhs=lgT[:],
                                  start=(mc == 0), stop=(mc == NMC - 1))

                  if stage in ("dma", "axpy"):
                      nc.vector.memset(acc_u[:], 0.0)
                      nc.vector.memset(acc_h[:], 0.0)
                      nc.sync.dma_start(out_u[:], acc_u[:])
                      nc.sync.dma_start(out_h[:], acc_h[:])
                      continue

                  # ---- Phase B: y^T = x^T - (Lx)^T; sn; stage AllGather ----
                  with (
                      tc.tile_pool(name=f"sqB{rep}", bufs=2) as sqB,
                      tc.tile_pool(name=f"psB{rep}", bufs=2,
                                   space="PSUM") as psB,
                  ):
                      snp = psB.tile([1, R], FP32, name=f"snp{rep}")
                      for fc in range(NFC):
                          nc.vector.scalar_tensor_tensor(
                              yT_own[fc], ytpsum[fc][:], -1.0,
                              xT_sb[:, fc, :], OP.mult, OP.add)
                          sq = sqB.tile([128, R], BF16, tag="sq")
                          nc.scalar.activation(sq[:], yT_own[fc], AF.Square)
                          nc.tensor.matmul(
                              snp[:], lhsT=ones_col[:], rhs=sq[:],
                              start=(fc == 0), stop=(fc == NFC - 1))
                      nc.sync.dma_start(
                          agin[0:F, :].rearrange("(fc p) n -> p fc n", p=128),
                          yT_all[:])
                      nc.scalar.copy(sn_sb[:], snp[:])
                      sn8_sb = sqB.tile([1, R], FP8, tag="sn8")
                      nc.scalar.activation(sn8_sb[:], snp[:], AF.Copy,
                                           scale=1.0 / 32.0)
                      nc.sync.dma_start(agin[F:F + 1, :], sn8_sb[:])
                      # sn columns [128,1] per own rc chunk (1-contraction MM)
                      onesp = sqB.tile([1, 1], BF16, tag="o1")
                      nc.vector.tensor_copy(onesp[:], ones_f[:, 0:1])
                      for rc in range(NRC):
                          snc = psB.tile([128, 1], FP32, tag="snc")
                          nc.tensor.matmul(
                              snc[:],
                              lhsT=sn_sb[0:1, rc * 128:(rc + 1) * 128],
                              rhs=onesp[:], start=True, stop=True)
                          nc.scalar.copy(sn_own[rc][:], snc[:])

              if stage == "A":
                  nc.sync.dma_start(out_u[:], acc_u[:])
                  nc.sync.dma_start(out_h[:], acc_h[:])
                  continue

              # ---------------- AllGather y_hat^T + sn ----------------
              if stage in ("sim", "simAG"):
                  # TimelineSim can't run collectives: stand in DMAs with
                  # equivalent traffic.
                  for r in range(CORES):
                      nc.sync.dma_start(agout[r], agin[:])
              else:
                  nc.gpsimd.collective_compute(
                      "AllGather", OP.bypass,
                      replica_groups=[list(range(CORES))],
                      ins=[agin[:]], outs=[agout[0:CORES]])

              if stage in ("AG", "simAG"):
                  nc.sync.dma_start(out_u[:], acc_u[:])
                  nc.sync.dma_start(out_h[:], acc_h[:])
                  continue

              # ---------------- Phase D: distance tiles + weighted sums -----
              sp_eng = nc.engines[mybir.EngineType.SP]
              pid = sp_eng.partition_id()
              rot = []  # SP registers holding (pid + j) % CORES for j >= 1
              for j in range(1, JBLK):
                  rj = sp_eng.alloc_register(f"rot{rep}_{j}")
                  sp_eng.reg_alu(rj, pid, j, OP.add)
                  sp_eng.reg_alu(rj, rj, CORES, OP.mod)
                  rot.append(bass.make_scalar_value(rj, min_val=0,
                                                    max_val=CORES - 1))
              with (
                  tc.tile_pool(name=f"yTD{rep}", bufs=1) as ytd_pool,
                  tc.tile_pool(name=f"snD{rep}", bufs=1) as sn_pool,
                  tc.tile_pool(name=f"sqD{rep}", bufs=4) as sqd_pool,
                  tc.tile_pool(name=f"dD{rep}", bufs=4) as dd_pool,
                  tc.tile_pool(name=f"ttD{rep}", bufs=2) as tt_pool,
                  tc.tile_pool(name=f"psG{rep}", bufs=3, space="PSUM") as psG,
                  tc.tile_pool(name=f"psV{rep}", bufs=2, space="PSUM") as psV,
                  tc.tile_pool(name=f"psS{rep}", bufs=1, space="PSUM") as psS,
              ):
                  # j-major loads so the j=1 block lands first; one DMA per
                  # remote block instead of one per fc chunk
                  yT_rot = ytd_pool.tile([128, JBLK - 1, NFC, R], FP8,
                                         name=f"yT_rot{rep}")
                  sn_rot = sn_pool.tile([1, JBLK - 1, R], FP8)
                  for j in range(1, JBLK):
                      nc.sync.dma_start(
                          yT_rot[:, j - 1, :, :],
                          agout[bass.ds(rot[j - 1], 1), 0:F, :]
                          .rearrange("r (fc p) n -> p (r fc) n", p=128))
                      nc.sync.dma_start(
                          sn_rot[:, j - 1, :],
                          agout[bass.ds(rot[j - 1], 1), F:F + 1, :]
                          .rearrange("r one n -> one (r n)"))
                  # broadcast sn rows to [128, R] per j block
                  sncol = sn_pool.tile([128, JBLK, R], FP32)
                  for j in range(JBLK):
                      snb = psS.tile([128, R], FP32, tag="snb")
                      # j=0: exact bf16 own sn; j>=1: fp8 sn/32, un-scaled
                      # by the 32-valued broadcast lhsT
                      if j == 0:
                          nc.tensor.matmul(snb[:], lhsT=ones_row[:],
                                           rhs=sn_sb[:],
                                           start=True, stop=True)
                      else:
                          nc.tensor.matmul(snb[:], lhsT=t32_row[:],
                                           rhs=sn_rot[:, j - 1, :],
                                           start=True, stop=True)
                      nc.scalar.copy(sncol[:, j, :], snb[:])

                  tiles = [(j, rc) for j in range(JBLK) for rc in range(NRC)]
                  vu = vh = None
                  pending = None  # (j, rc, d_tile) awaiting V matmuls

                  def flush_pending():
                      nonlocal pending
                      if pending is None:
                          return
                      pj, prc, pdt = pending
                      nc.tensor.matmul(
                          vu[:], lhsT=uT_sb[:, prc, :], rhs=pdt[:],
                          start=(prc == 0), stop=(prc == NRC - 1))
                      nc.tensor.matmul(
                          vh[:], lhsT=hT_sb[:, prc, :], rhs=pdt[:],
                          start=(prc == 0), stop=(prc == NRC - 1))
                      pending = None
                      if prc == NRC - 1:
                          su = tt_pool.tile([NCLS, R], FP32, tag="su",
                                            name=f"su{rep}_{pj}")
                          nc.vector.tensor_tensor(
                              out=su[:], in0=vu[:], in1=u_sb[:, pj, :],
                              op=OP.mult)
                          nc.vector.reduce_sum(
                              acc_u[:, pj:pj + 1], su[:],
                              axis=mybir.AxisListType.X)
                          sh = tt_pool.tile([G, R], FP32, tag="sh",
                                            name=f"sh{rep}_{pj}")
                          nc.vector.tensor_tensor(
                              out=sh[:], in0=vh[:], in1=h_sb[:, pj, :],
                              op=OP.mult)
                          nc.vector.reduce_sum(
                              acc_h[:, pj:pj + 1], sh[:],
                              axis=mybir.AxisListType.X)

                  for j, rc in tiles:
                      if rc == 0:
                          new_vu = psV.tile([NCLS, R], FP32, tag="vu",
                                            name=f"vu{rep}_{j}")
                          new_vh = psV.tile([G, R], FP32, tag="vh",
                                            name=f"vh{rep}_{j}")
                      gram = psG.tile([128, R], FP32, tag="g",
                                      name=f"gram{rep}_{j}_{rc}")
                      for fc in range(NFC):
                          rhs = (yT_own[fc] if j == 0
                                 else yT_rot[:, j - 1, fc, :])
                          nc.tensor.matmul(
                              gram[:],
                              lhsT=yT_all[:, fc, rc * 128:(rc + 1) * 128],
                              rhs=rhs,
                              start=(fc == 0), stop=(fc == NFC - 1))
                      flush_pending()
                      if rc == 0:
                          vu, vh = new_vu, new_vh
                      sq = sqd_pool.tile([128, R], FP32, tag="sq")
                      nc.vector.scalar_tensor_tensor(
                          sq[:], gram[:], -2.0, sncol[:, j, :],
                          OP.mult, OP.add)
                      nc.vector.tensor_scalar(
                          sq[:], sq[:], sn_own[rc][:], 0.0, OP.add, OP.max)
                      dt = dd_pool.tile([128, R], BF16, tag="d")
                      nc.scalar.activation(dt[:], sq[:], AF.Sqrt)
                      if j == 0:
                          nc.vector.tensor_tensor(
                              out=dt[:], in0=dt[:], in1=dmask[:, rc, :],
                              op=OP.mult)
                      pending = (j, rc, dt)
                  flush_pending()

                  nc.sync.dma_start(out_u[:], acc_u[:])
                  nc.sync.dma_start(out_h[:], acc_h[:])
                  if chained:
                      nc.sync.dma_start(chain[:], acc_u[0:1, 0:1])

    nc.compile()
    return nc


def host_prep(cfg, D, x, C, mask, y, groups):
    """Host-side input prep: normalize C, build weight matrices, shard."""
    c = _derived(cfg)
    N, K, G, NCLS, CORES, R = c["N"], c["K"], c["G"], c["NCLS"], c["CORES"], c["R"]
    NRC, JBLK = c["NRC"], c["JBLK"]
    bf16 = mybir.dt.np(BF16)

    C32 = np.asarray(C, np.float32)
    cn = (C32 / np.linalg.norm(C32, axis=0, keepdims=True)).astype(np.float32)
    dim = np.float32(math.sqrt(K))
    nrm = np.linalg.norm(cn, axis=0).astype(np.float32)
    sparsity = float(np.mean((dim - np.abs(cn).sum(0) / nrm) / (dim - 1.0)))

    mask_b = np.asarray(mask, bool)
    y_i = np.asarray(y, np.int64)
    cnt = np.zeros(NCLS, np.int64)
    np.add.at(cnt, y_i[mask_b], 1)
    u = np.zeros((NCLS, N), np.float32)
    sel = mask_b & (cnt[y_i] > 0)
    u[y_i[sel], np.nonzero(sel)[0]] = 1.0 / cnt[y_i[sel]]

    g_i = np.asarray(groups, np.int64)
    H = np.zeros((G, N), np.float32)
    np.add.at(H, (np.repeat(np.arange(G), g_i.shape[1]), g_i.ravel()), 1.0)

    uT = np.ascontiguousarray(u.T).astype(bf16)
    hT = np.ascontiguousarray(H.T).astype(bf16)
    x32 = np.ascontiguousarray(np.asarray(x, np.float32))
    x16 = x32.astype(bf16)
    D32 = np.asarray(D, np.float32)

    # D is shipped as fp8e4m3, pre-scaled by a power of two chosen so the
    # largest |D| sits near the top of the fp8 range; the inverse scale is
    # folded into the cn coefficients so the kernel math is unchanged.
    fp8 = mybir.dt.np(FP8)
    dmax = float(np.abs(D32).max())
    dscale = float(2.0 ** math.floor(math.log2(224.0 / max(dmax, 1e-30))))
    cnb = np.tile((cn / dscale).ravel()[None, :], (128, 1)).astype(np.float32)

    # diagonal mask for the j=0 (own) block: 0 at global col == global row
    dmask = np.ones((128, NRC, R), np.float32)
    for rc in range(NRC):
        for p in range(128):
            dmask[p, rc, rc * 128 + p] = 0.0
    dmask = dmask.astype(bf16)

    in_maps = []
    for ci in range(CORES):
        sl = slice(ci * R, (ci + 1) * R)
        # rotated, symmetry-scaled weight slices: j -> global block (ci+j)%CORES
        u_rot = np.zeros((NCLS, JBLK, R), np.float32)
        h_rot = np.zeros((G, JBLK, R), np.float32)
        for j in range(JBLK):
            gb = (ci + j) % CORES
            scale = 1.0 if j == 0 else 2.0
            if j == CORES // 2 and ci >= CORES // 2:
                continue  # pair already handled by core ci - CORES//2
            u_rot[:, j, :] = u[:, gb * R:(gb + 1) * R] * scale
            h_rot[:, j, :] = H[:, gb * R:(gb + 1) * R] * scale
        in_maps.append({
            # [R, N, K] -> [N, K, R] (m-major, k outer, own-rows inner)
            "Dsh": np.ascontiguousarray(
                (D32[sl] * dscale).transpose(1, 2, 0)).astype(fp8),
            "x_in": x32.astype(fp8),
            "xT_own": np.ascontiguousarray(x32[sl].T).astype(bf16),
            "cnb": cnb,
            "uT_sh": np.ascontiguousarray(uT[sl]),
            "hT_sh": np.ascontiguousarray(hT[sl]),
            "u_rot": u_rot,
            "h_rot": h_rot,
            "dmask": dmask,
        })
    return in_maps, sparsity


def combine(cfg, results, sparsity, group_size):
    """loss = sparsity + hl2 + hl1/beta, from per-core partial sums."""
    beta = np.float64(cfg["G"]) / np.float64(cfg["NCLS"])
    hl2 = np.float64(0.0)
    s1 = np.float64(0.0)
    for r in results:
        hl2 += r["out_u"].astype(np.float64).sum()
        s1 += r["out_h"].astype(np.float64).sum()
    hl1 = -s1 / np.float64(group_size * group_size)
    total = np.float64(sparsity) + hl2 + hl1 / beta
    return np.float32(total)


_BUILD_CACHE = {}


def _get_nc(key, cfg):
    if key not in _BUILD_CACHE:
        _BUILD_CACHE[key] = build(cfg)
    return _BUILD_CACHE[key]


def kernel(D, x, C, mask, y, groups):
    cfg = dict(FULL_CFG)
    in_maps, sparsity = host_prep(cfg, D, x, C, mask, y, groups)
    nc = _get_nc("full", cfg)
    res = run_bass_kernel_spmd(
        nc, in_maps, core_ids=list(range(cfg["CORES"])), trace=False)
    return combine(cfg, res.results, sparsity, np.asarray(groups).shape[1])



# revision 25
# speedup vs baseline: 1.0915x; 1.0915x over previous
"""Trainium2 Bass kernel for nn_DictNet loss (8-core SPMD), v11.

v11 = v10 + coalesced collective staging + fp8 x
------------------------------------------------
* The AllGather staging write is ONE DMA (single [128, NFC, R] y^T
  tile) and each rotated remote block loads with ONE DMA instead of
  one per feature chunk: ~15 fixed DMA costs come off the serialized
  post-collective tail.
* x is shipped fp8 for the (Lx)^T matmul lhsT (the direct x^T
  subtraction stays bf16), halving the resident x load.
* L^T ring deepened to 5 buffers so the AXPY never stalls on matmul
  consumers. (Phase-D PSUM is exactly full at 8 banks: psG 3 + psV 2
  pairs + psS 1 — no headroom there.)

v10 = v9b + fp8-resident D stream
---------------------------------
* D tiles stay fp8 in SBUF (no DMA cast): SBUF ingress for the D
  stream halves again and the prefetch ring deepens (12 x 720 KB). The
  AXPY reads the fp8 tiles directly (DVE converts in-pipe); its bf16
  accumulator is unchanged. D DMAs stay on the GpSimd (SWDGE) queue so
  x/weight loads on the HWDGE queue aren't serialized behind them.

v9b = v8 + fp8 AllGather payload
--------------------------------
* y_hat^T is written in fp8e4m3: the AllGather payload, the rotated
  y^T tiles and both gram operands halve. sn rides the collective as a
  /32-scaled fp8 row, un-scaled by a 32-valued broadcast lhsT; the own
  (row-side) sn stays fp32-exact.

v8 = v7 + host-transposed D (direct L^T production)
---------------------------------------------------
* D is shipped pre-transposed per core as D^T[m, k, r] so the AXPY
  produces L^T chunks [128 m, R] directly: the whole identity-matmul
  L -> L^T fold, its PSUM traffic and the ACT copy back to SBUF are
  deleted. The y^T matmuls consume the AXPY output tile as rhs as-is.
* k is outermost in the tile free dims, so every AXPY term reads a
  contiguous 512-wide bf16 row (16-bit 2x DVE mode eligible), instead
  of the stride-11 reads of v6/v7.

v7 = v6 + fp8 D in HBM
----------------------
* D is pre-scaled by a power of two (folded into cn, so the math is
  unchanged) and shipped to the device as fp8e4m3: HBM reads of the D
  stream halve vs bf16 (quarter vs the original fp32). The SWDGE DMA
  casts fp8 -> bf16 on the way into SBUF, so everything downstream is
  identical to v6.


Math restructuring (same as v1)
-------------------------------
  Cn    = C / ||C||                      (tiny, host)
  L     = einsum('nmk,k->nm', D, Cn)     (memory-bound: 738 MB of D)
  y_hat = x - L @ x
  d     = pairwise distance matrix of y_hat rows   [N, N]
  loss  = sparsity(Cn) + sum_c u_c d u_c^T - (1/(S^2*beta)) * sum_g h_g d h_g^T

v6 = v2 + bf16-cast D stream
----------------------------
* D tiles are cast fp32 -> bf16 during the DMA (SWDGE cast path): HBM reads
  are unchanged but the DVE AXPY (the hardware phase-A bottleneck) runs at
  the 2x 16-bit rate; the L^T fold is a regular matmul against a bf16
  identity so the PSUM stays fp32 (plain TRN2 ISA).

v2 performance restructure
--------------------------
* y_hat is accumulated TRANSPOSED (y^T[f, own-rows]) directly in PSUM by
  swapping the matmul operands (lhsT = x block, rhs = L^T block), which
  deletes the whole post-phase-A transpose stage.
* x is loaded once (bf16, SBUF-resident) instead of re-streamed per m-group.
* Everything downstream of y_hat is bf16: the AllGather payload, the y^T
  tiles, the gram/vu/vh matmul operands and the distance tiles. PSUM math
  stays fp32.
* All small phase-D weights are DMA'd during the D stream (front-loaded).
* The last m-group is split into two 256-wide groups so the post-DMA AXPY
  drain is short.

Sharding: D rows (node axis) split across 8 cores; y_hat^T AllGathered so
every core forms distance tiles for its own rows. Symmetry: each core only
processes JBLK = CORES/2 + 1 rotated column blocks; off-diagonal blocks are
double-counted via host-scaled weights; the j = CORES/2 block is
zero-weighted on the upper half of the cores.
"""

import math

import numpy as np

import concourse.bass as bass
import concourse.mybir as mybir
import concourse.tile as tile
from concourse import bacc
from concourse.bass_utils import run_bass_kernel_spmd

FP32 = mybir.dt.float32
BF16 = mybir.dt.bfloat16
FP8 = mybir.dt.float8e4
AF = mybir.ActivationFunctionType
OP = mybir.AluOpType

FULL_CFG = dict(N=4096, F=512, K=11, G=128, NCLS=7, CORES=8)


def _derived(cfg):
    N, F, K, G, NCLS, CORES = (
        cfg["N"], cfg["F"], cfg["K"], cfg["G"], cfg["NCLS"], cfg["CORES"])
    R = N // CORES              # rows per core
    assert R % 128 == 0 and N % 512 == 0 and F % 128 == 0
    NRC = R // 128              # 128-row chunks per core
    NMC = N // 128              # 128-col chunks (m axis)
    NFC = F // 128              # feature chunks
    XSUB = N // 128             # m sub-blocks in resident x
    JBLK = CORES // 2 + 1       # rotated col blocks each core processes
    return dict(N=N, F=F, K=K, G=G, NCLS=NCLS, CORES=CORES, R=R, NRC=NRC,
                NMC=NMC, NFC=NFC, XSUB=XSUB, JBLK=JBLK)


def build(cfg, reps=1, stage="full", chained=False):
    """Build the SPMD kernel (one NEFF, runs on all cores).

    reps > 1 repeats the whole computation serially (timing probe).
    stage: "dma" = D loads only, "axpy" = + AXPY, "A" = phases A+B,
    "AG"/"simAG" = + collective (simAG fakes it), "sim" = full with faked
    collective (for TimelineSim), "full" = everything.
    """
    c = _derived(cfg)
    N, F, K, G, NCLS = c["N"], c["F"], c["K"], c["G"], c["NCLS"]
    CORES, R, NRC, NMC = c["CORES"], c["R"], c["NRC"], c["NMC"]
    NFC, XSUB, JBLK = c["NFC"], c["XSUB"], c["JBLK"]

    nc = bacc.Bacc("TRN2", target_bir_lowering=False, debug=False,
                   num_devices=CORES)

    # ---- I/O ----
    # D^T per core: Dsh[m, k, r] = D[r, m, k] (host pre-transposed)
    Dsh = nc.dram_tensor("Dsh", [N, K, R], FP8, kind="ExternalInput")
    x_in = nc.dram_tensor("x_in", [N, F], FP8, kind="ExternalInput")
    xT_in = nc.dram_tensor("xT_own", [F, R], BF16, kind="ExternalInput")
    cnb_in = nc.dram_tensor("cnb", [128, K], FP32, kind="ExternalInput")
    uT_in = nc.dram_tensor("uT_sh", [R, NCLS], BF16, kind="ExternalInput")
    hT_in = nc.dram_tensor("hT_sh", [R, G], BF16, kind="ExternalInput")
    u_in = nc.dram_tensor("u_rot", [NCLS, JBLK, R], FP32, kind="ExternalInput")
    h_in = nc.dram_tensor("h_rot", [G, JBLK, R], FP32, kind="ExternalInput")
    dmask_in = nc.dram_tensor("dmask", [128, NRC, R], BF16, kind="ExternalInput")
    out_u = nc.dram_tensor("out_u", [NCLS, JBLK], FP32, kind="ExternalOutput")
    out_h = nc.dram_tensor("out_h", [G, JBLK], FP32, kind="ExternalOutput")

    # cross-rep serialization bounce for single-shot timing (chained=True)
    chain = nc.dram_tensor("chain", [1, 1], FP32)
    # collective bounce buffers: rows 0..F-1 = y_hat^T (own cols, fp8),
    # row F = sn/32 (fp8)
    agin = nc.dram_tensor("agin", [F + 1, R], FP8)
    agout = nc.dram_tensor("agout", [CORES, F + 1, R], FP8,
                           addr_space="Shared")

    with tile.TileContext(nc) as tc:
      for rep in range(reps):
          with tc.tile_pool(name=f"persist{rep}", bufs=1) as pp:
              cnb = pp.tile([128, K], FP32)
              nc.sync.dma_start(cnb[:], cnb_in[:])
              if chained and rep > 0:
                  # rep k's first consumer waits on rep k-1's last result:
                  # cnb[0,0] = 0*chain + cnb[0,0] forces the dependency
                  # through real dataflow without changing the value
                  cht = pp.tile([1, 1], FP32, name=f"cht{rep}")
                  nc.sync.dma_start(cht[:], chain[:])
                  nc.vector.scalar_tensor_tensor(
                      cnb[0:1, 0:1], cht[:], 0.0, cnb[0:1, 0:1],
                      OP.mult, OP.add)

              # constants: ones in bf16 (memset fp32 then cast-copy)
              ones_f = pp.tile([1, 128], FP32)
              nc.vector.memset(ones_f[:], 1.0)
              ones_row = pp.tile([1, 128], BF16)   # [1,128] lhsT broadcaster
              nc.vector.tensor_copy(ones_row[:], ones_f[:])
              t32_f = pp.tile([1, 128], FP32)
              nc.vector.memset(t32_f[:], 32.0)
              t32_row = pp.tile([1, 128], BF16)    # un-scales the fp8 sn row
              nc.vector.tensor_copy(t32_row[:], t32_f[:])
              onesc_f = pp.tile([128, 1], FP32)
              nc.vector.memset(onesc_f[:], 1.0)
              ones_col = pp.tile([128, 1], BF16)   # [128,1] column reducer
              nc.vector.tensor_copy(ones_col[:], onesc_f[:])

              # single tile holding all fc chunks: the AllGather staging
              # write is ONE DMA instead of NFC
              yT_all = pp.tile([128, NFC, R], FP8, name=f"yT_all{rep}")
              yT_own = [yT_all[:, fc, :] for fc in range(NFC)]
              sn_own = [pp.tile([128, 1], FP32, tag=f"sn{rc}",
                                name=f"sn_own{rep}_{rc}")
                        for rc in range(NRC)]
              sn_sb = pp.tile([1, R], BF16, name=f"sn_sb{rep}")
              acc_u = pp.tile([NCLS, JBLK], FP32)
              acc_h = pp.tile([G, JBLK], FP32)
              if stage not in ("full", "sim"):
                  nc.vector.memset(acc_u[:], 0.0)
                  nc.vector.memset(acc_h[:], 0.0)

              # ------------- Phase A: L^T = sum_k cn_k * D^T_k; yT -= (Lx)^T
              with (
                  tc.tile_pool(name=f"psYT{rep}", bufs=1, space="PSUM") as psYT,
              ):
                  ytpsum = [psYT.tile([128, R], FP32, tag=f"ytp{fc}",
                                      name=f"ytpsum{rep}_{fc}")
                            for fc in range(NFC)]
                  with (
                      tc.tile_pool(name=f"dA{rep}", bufs=12) as dpool,
                      tc.tile_pool(name=f"lA{rep}", bufs=5) as lpool,
                  ):
                      # software-pipelined D-tile DMA issue: the queue is
                      # FIFO, so the big x load and the small phase-B/D
                      # weights slot in behind the first D chunks instead of
                      # delaying them
                      dtile = {}
                      issued = [0]

                      def issue_d(n):
                          for _ in range(n):
                              if issued[0] >= NMC:
                                  return
                              mc = issued[0]
                              t = dpool.tile([128, K, R], FP8, tag="D")
                              nc.gpsimd.dma_start(
                                  t[:], Dsh[mc * 128:(mc + 1) * 128, :, :])
                              dtile[mc] = t
                              issued[0] += 1

                      issue_d(4)  # first chunks ahead of everything else
                      # resident x (bf16): [p, m-sub, f]
                      x_sb = pp.tile([128, XSUB, F], FP8, name=f"x_sb{rep}")
                      nc.sync.dma_start(
                          x_sb[:], x_in[:].rearrange("(s p) f -> p s f", p=128))
                      issue_d(2)
                      # small phase-B/D operands, loaded under the D stream
                      xT_sb = pp.tile([128, NFC, R], BF16, name=f"xT_sb{rep}")
                      nc.sync.dma_start(
                          xT_sb[:],
                          xT_in[:].rearrange("(fc p) n -> p fc n", p=128))
                      uT_sb = pp.tile([128, NRC, NCLS], BF16,
                                      name=f"uT_sb{rep}")
                      nc.sync.dma_start(
                          uT_sb[:],
                          uT_in[:].rearrange("(rc p) c -> p rc c", p=128))
                      hT_sb = pp.tile([128, NRC, G], BF16, name=f"hT_sb{rep}")
                      nc.sync.dma_start(
                          hT_sb[:],
                          hT_in[:].rearrange("(rc p) g -> p rc g", p=128))
                      u_sb = pp.tile([NCLS, JBLK, R], FP32, name=f"u_sb{rep}")
                      nc.sync.dma_start(u_sb[:], u_in[:])
                      h_sb = pp.tile([G, JBLK, R], FP32, name=f"h_sb{rep}")
                      nc.sync.dma_start(h_sb[:], h_in[:])
                      dmask = pp.tile([128, NRC, R], BF16, name=f"dmask{rep}")
                      nc.sync.dma_start(dmask[:], dmask_in[:])

                      junk = pp.tile([128, 1], BF16, name=f"junk{rep}")
                      for mc in range(NMC):
                          issue_d(1)
                          if stage == "dma":
                              # tiny consumer so the DMA can't be elided
                              nc.vector.tensor_copy(
                                  junk[:], dtile.pop(mc)[:, 0, 0:1])
                              continue
                          dt = dtile.pop(mc)
                          lgT = lpool.tile([128, R], BF16, tag="L",
                                           name=f"lgT{rep}_{mc}")
                          nc.vector.tensor_scalar_mul(
                              lgT[:], dt[:, 0, :], cnb[:, 0:1])
                          for k in range(1, K):
                              nc.vector.scalar_tensor_tensor(
                                  lgT[:], dt[:, k, :],
                                  cnb[:, k:k + 1], lgT[:],
                                  OP.mult, OP.add)
                          if stage == "axpy":
                              nc.vector.tensor_copy(junk[:], lgT[:, 0:1])
                              continue
                          for fc in range(NFC):
                              nc.tensor.matmul(
                                  ytpsum[fc][:],
                                  lhsT=x_sb[:, mc, fc * 128:(fc + 1) * 128],
                                  rhs=lgT[:],
                                  start=(mc == 0), stop=(mc == NMC - 1))

                  if stage in ("dma", "axpy"):
                      nc.vector.memset(acc_u[:], 0.0)
                      nc.vector.memset(acc_h[:], 0.0)
                      nc.sync.dma_start(out_u[:], acc_u[:])
                      nc.sync.dma_start(out_h[:], acc_h[:])
                      continue

                  # ---- Phase B: y^T = x^T - (Lx)^T; sn; stage AllGather ----
                  with (
                      tc.tile_pool(name=f"sqB{rep}", bufs=2) as sqB,
                      tc.tile_pool(name=f"psB{rep}", bufs=2,
                                   space="PSUM") as psB,
                  ):
                      snp = psB.tile([1, R], FP32, name=f"snp{rep}")
                      for fc in range(NFC):
                          nc.vector.scalar_tensor_tensor(
                              yT_own[fc], ytpsum[fc][:], -1.0,
                              xT_sb[:, fc, :], OP.mult, OP.add)
                          sq = sqB.tile([128, R], BF16, tag="sq")
                          nc.scalar.activation(sq[:], yT_own[fc], AF.Square)
                          nc.tensor.matmul(
                              snp[:], lhsT=ones_col[:], rhs=sq[:],
                              start=(fc == 0), stop=(fc == NFC - 1))
                      nc.sync.dma_start(
                          agin[0:F, :].rearrange("(fc p) n -> p fc n", p=128),
                          yT_all[:])
                      nc.scalar.copy(sn_sb[:], snp[:])
                      sn8_sb = sqB.tile([1, R], FP8, tag="sn8")
                      nc.scalar.activation(sn8_sb[:], snp[:], AF.Copy,
                                           scale=1.0 / 32.0)
                      nc.sync.dma_start(agin[F:F + 1, :], sn8_sb[:])
                      # sn columns [128,1] per own rc chunk (1-contraction MM)
                      onesp = sqB.tile([1, 1], BF16, tag="o1")
                      nc.vector.tensor_copy(onesp[:], ones_f[:, 0:1])
                      for rc in range(NRC):
                          snc = psB.tile([128, 1], FP32, tag="snc")
                          nc.tensor.matmul(
                              snc[:],
                              lhsT=sn_sb[0:1, rc * 128:(rc + 1) * 128],
                              rhs=onesp[:], start=True, stop=True)
                          nc.scalar.copy(sn_own[rc][:], snc[:])

              if stage == "A":
                  nc.sync.dma_start(out_u[:], acc_u[:])
                  nc.sync.dma_start(out_h[:], acc_h[:])
                  continue

              # ---------------- AllGather y_hat^T + sn ----------------
              if stage in ("sim", "simAG"):
                  # TimelineSim can't run collectives: stand in DMAs with
                  # equivalent traffic.
                  for r in range(CORES):
                      nc.sync.dma_start(agout[r], agin[:])
              else:
                  nc.gpsimd.collective_compute(
                      "AllGather", OP.bypass,
                      replica_groups=[list(range(CORES))],
                      ins=[agin[:]], outs=[agout[0:CORES]])

              if stage in ("AG", "simAG"):
                  nc.sync.dma_start(out_u[:], acc_u[:])
                  nc.sync.dma_start(out_h[:], acc_h[:])
                  continue

              # ---------------- Phase D: distance tiles + weighted sums -----
              sp_eng = nc.engines[mybir.EngineType.SP]
              pid = sp_eng.partition_id()
              rot = []  # SP registers holding (pid + j) % CORES for j >= 1
              for j in range(1, JBLK):
                  rj = sp_eng.alloc_register(f"rot{rep}_{j}")
                  sp_eng.reg_alu(rj, pid, j, OP.add)
                  sp_eng.reg_alu(rj, rj, CORES, OP.mod)
                  rot.append(bass.make_scalar_value(rj, min_val=0,
                                                    max_val=CORES - 1))
              with (
                  tc.tile_pool(name=f"yTD{rep}", bufs=1) as ytd_pool,
                  tc.tile_pool(name=f"snD{rep}", bufs=1) as sn_pool,
                  tc.tile_pool(name=f"sqD{rep}", bufs=4) as sqd_pool,
                  tc.tile_pool(name=f"dD{rep}", bufs=4) as dd_pool,
                  tc.tile_pool(name=f"ttD{rep}", bufs=2) as tt_pool,
                  tc.tile_pool(name=f"psG{rep}", bufs=3, space="PSUM") as psG,
                  tc.tile_pool(name=f"psV{rep}", bufs=2, space="PSUM") as psV,
                  tc.tile_pool(name=f"psS{rep}", bufs=1, space="PSUM") as psS,
              ):
                  # j-major loads so the j=1 block lands first; one DMA per
                  # remote block instead of one per fc chunk
                  yT_rot = ytd_pool.tile([128, JBLK - 1, NFC, R], FP8,
                                         name=f"yT_rot{rep}")
                  sn_rot = sn_pool.tile([1, JBLK - 1, R], FP8)
                  for j in range(1, JBLK):
                      nc.sync.dma_start(
                          yT_rot[:, j - 1, :, :],
                          agout[bass.ds(rot[j - 1], 1), 0:F, :]
                          .rearrange("r (fc p) n -> p (r fc) n", p=128))
                      nc.sync.dma_start(
                          sn_rot[:, j - 1, :],
                          agout[bass.ds(rot[j - 1], 1), F:F + 1, :]
                          .rearrange("r one n -> one (r n)"))
                  # broadcast sn rows to [128, R] per j block
                  sncol = sn_pool.tile([128, JBLK, R], FP32)
                  for j in range(JBLK):
                      snb = psS.tile([128, R], FP32, tag="snb")
                      # j=0: exact bf16 own sn; j>=1: fp8 sn/32, un-scaled
                      # by the 32-valued broadcast lhsT
                      if j == 0:
                          nc.tensor.matmul(snb[:], lhsT=ones_row[:],
                                           rhs=sn_sb[:],
                                           start=True, stop=True)
                      else:
                          nc.tensor.matmul(snb[:], lhsT=t32_row[:],
                                           rhs=sn_rot[:, j - 1, :],
                                           start=True, stop=True)
                      nc.scalar.copy(sncol[:, j, :], snb[:])

                  tiles = [(j, rc) for j in range(JBLK) for rc in range(NRC)]
                  vu = vh = None
                  pending = None  # (j, rc, d_tile) awaiting V matmuls

                  def flush_pending():
                      nonlocal pending
                      if pending is None:
                          return
                      pj, prc, pdt = pending
                      nc.tensor.matmul(
                          vu[:], lhsT=uT_sb[:, prc, :], rhs=pdt[:],
                          start=(prc == 0), stop=(prc == NRC - 1))
                      nc.tensor.matmul(
                          vh[:], lhsT=hT_sb[:, prc, :], rhs=pdt[:],
                          start=(prc == 0), stop=(prc == NRC - 1))
                      pending = None
                      if prc == NRC - 1:
                          su = tt_pool.tile([NCLS, R], FP32, tag="su",
                                            name=f"su{rep}_{pj}")
                          nc.vector.tensor_tensor(
                              out=su[:], in0=vu[:], in1=u_sb[:, pj, :],
                              op=OP.mult)
                          nc.vector.reduce_sum(
                              acc_u[:, pj:pj + 1], su[:],
                              axis=mybir.AxisListType.X)
                          sh = tt_pool.tile([G, R], FP32, tag="sh",
                                            name=f"sh{rep}_{pj}")
                          nc.vector.tensor_tensor(
                              out=sh[:], in0=vh[:], in1=h_sb[:, pj, :],
                              op=OP.mult)
                          nc.vector.reduce_sum(
                              acc_h[:, pj:pj + 1], sh[:],
                              axis=mybir.AxisListType.X)

                  for j, rc in tiles:
                      if rc == 0:
                          new_vu = psV.tile([NCLS, R], FP32, tag="vu",
                                            name=f"vu{rep}_{j}")
                          new_vh = psV.tile([G, R], FP32, tag="vh",
                                            name=f"vh{rep}_{j}")
                      gram = psG.tile([128, R], FP32, tag="g",
                                      name=f"gram{rep}_{j}_{rc}")
                      for fc in range(NFC):
                          rhs = (yT_own[fc] if j == 0
                                 else yT_rot[:, j - 1, fc, :])
                          nc.tensor.matmul(
                              gram[:],
                              lhsT=yT_all[:, fc, rc * 128:(rc + 1) * 128],
                              rhs=rhs,
                              start=(fc == 0), stop=(fc == NFC - 1))
                      flush_pending()
                      if rc == 0:
                          vu, vh = new_vu, new_vh
                      sq = sqd_pool.tile([128, R], FP32, tag="sq")
                      nc.vector.scalar_tensor_tensor(
                          sq[:], gram[:], -2.0, sncol[:, j, :],
                          OP.mult, OP.add)
                      nc.vector.tensor_scalar(
                          sq[:], sq[:], sn_own[rc][:], 0.0, OP.add, OP.max)
                      dt = dd_pool.tile([128, R], BF16, tag="d")
                      nc.scalar.activation(dt[:], sq[:], AF.Sqrt)
                      if j == 0:
                          nc.vector.tensor_tensor(
                              out=dt[:], in0=dt[:], in1=dmask[:, rc, :],
                              op=OP.mult)
                      pending = (j, rc, dt)
                  flush_pending()

                  nc.sync.dma_start(out_u[:], acc_u[:])
                  nc.sync.dma_start(out_h[:], acc_h[:])
                  if chained:
                      nc.sync.dma_start(chain[:], acc_u[0:1, 0:1])

    nc.compile()
    return nc


def host_prep(cfg, D, x, C, mask, y, groups):
    """Host-side input prep: normalize C, build weight matrices, shard."""
    c = _derived(cfg)
    N, K, G, NCLS, CORES, R = c["N"], c["K"], c["G"], c["NCLS"], c["CORES"], c["R"]
    NRC, JBLK = c["NRC"], c["JBLK"]
    bf16 = mybir.dt.np(BF16)

    C32 = np.asarray(C, np.float32)
    cn = (C32 / np.linalg.norm(C32, axis=0, keepdims=True)).astype(np.float32)
    dim = np.float32(math.sqrt(K))
    nrm = np.linalg.norm(cn, axis=0).astype(np.float32)
    sparsity = float(np.mean((dim - np.abs(cn).sum(0) / nrm) / (dim - 1.0)))

    mask_b = np.asarray(mask, bool)
    y_i = np.asarray(y, np.int64)
    cnt = np.zeros(NCLS, np.int64)
    np.add.at(cnt, y_i[mask_b], 1)
    u = np.zeros((NCLS, N), np.float32)
    sel = mask_b & (cnt[y_i] > 0)
    u[y_i[sel], np.nonzero(sel)[0]] = 1.0 / cnt[y_i[sel]]

    g_i = np.asarray(groups, np.int64)
    H = np.zeros((G, N), np.float32)
    np.add.at(H, (np.repeat(np.arange(G), g_i.shape[1]), g_i.ravel()), 1.0)

    uT = np.ascontiguousarray(u.T).astype(bf16)
    hT = np.ascontiguousarray(H.T).astype(bf16)
    x32 = np.ascontiguousarray(np.asarray(x, np.float32))
    x16 = x32.astype(bf16)
    D32 = np.asarray(D, np.float32)

    # D is shipped as fp8e4m3, pre-scaled by a power of two chosen so the
    # largest |D| sits near the top of the fp8 range; the inverse scale is
    # folded into the cn coefficients so the kernel math is unchanged.
    fp8 = mybir.dt.np(FP8)
    dmax = float(np.abs(D32).max())
    dscale = float(2.0 ** math.floor(math.log2(224.0 / max(dmax, 1e-30))))
    cnb = np.tile((cn / dscale).ravel()[None, :], (128, 1)).astype(np.float32)

    # diagonal mask for the j=0 (own) block: 0 at global col == global row
    dmask = np.ones((128, NRC, R), np.float32)
    for rc in range(NRC):
        for p in range(128):
            dmask[p, rc, rc * 128 + p] = 0.0
    dmask = dmask.astype(bf16)

    in_maps = []
    for ci in range(CORES):
        sl = slice(ci * R, (ci + 1) * R)
        # rotated, symmetry-scaled weight slices: j -> global block (ci+j)%CORES
        u_rot = np.zeros((NCLS, JBLK, R), np.float32)
        h_rot = np.zeros((G, JBLK, R), np.float32)
        for j in range(JBLK):
            gb = (ci + j) % CORES
            scale = 1.0 if j == 0 else 2.0
            if j == CORES // 2 and ci >= CORES // 2:
                continue  # pair already handled by core ci - CORES//2
            u_rot[:, j, :] = u[:, gb * R:(gb + 1) * R] * scale
            h_rot[:, j, :] = H[:, gb * R:(gb + 1) * R] * scale
        in_maps.append({
            # [R, N, K] -> [N, K, R] (m-major, k outer, own-rows inner)
            "Dsh": np.ascontiguousarray(
                (D32[sl] * dscale).transpose(1, 2, 0)).astype(fp8),
            "x_in": x32.astype(fp8),
            "xT_own": np.ascontiguousarray(x32[sl].T).astype(bf16),
            "cnb": cnb,
            "uT_sh": np.ascontiguousarray(uT[sl]),
            "hT_sh": np.ascontiguousarray(hT[sl]),
            "u_rot": u_rot,
            "h_rot": h_rot,
            "dmask": dmask,
        })
    return in_maps, sparsity


def combine(cfg, results, sparsity, group_size):
    """loss = sparsity + hl2 + hl1/beta, from per-core partial sums."""
    beta = np.float64(cfg["G"]) / np.float64(cfg["NCLS"])
    hl2 = np.float64(0.0)
    s1 = np.float64(0.0)
    for r in results:
        hl2 += r["out_u"].astype(np.float64).sum()
        s1 += r["out_h"].astype(np.float64).sum()
    hl1 = -s1 / np.float64(group_size * group_size)
    total = np.float64(sparsity) + hl2 + hl1 / beta
    return np.float32(total)


_BUILD_CACHE = {}


def _get_nc(key, cfg):
    if key not in _BUILD_CACHE:
        _BUILD_CACHE[key] = build(cfg)
    return _BUILD_CACHE[key]


def kernel(D, x, C, mask, y, groups):
    cfg = dict(FULL_CFG)
    in_maps, sparsity = host_prep(cfg, D, x, C, mask, y, groups)
    nc = _get_nc("full", cfg)
    res = run_bass_kernel_spmd(
        nc, in_maps, core_ids=list(range(cfg["CORES"])), trace=False)
    return combine(cfg, res.results, sparsity, np.asarray(groups).shape[1])

